# revision 1
# baseline (speedup 1.0000x reference)
"""Trainium2 Bass kernel for an LSTM cell forecaster.

Model (PyTorch LSTMCell semantics, see reference):
  encode:   512 steps of LSTMCell over x[:, t, :] (input size 2, hidden 128)
  forecast: 50 steps where the input is y = fc(h) (output size 2)
  output:   concat of the 50 y's -> [B, 100]

Distribution: data-parallel over batch. B=4096 is split across 8 cores
(512 rows per core); the tiny weights are replicated. The recurrence over
time stays local to each core.

Per-core layout: hidden units on SBUF partitions, batch on the free dim.
Gate pre-activations live in PSUM as [128, 4(gate), 512] (4 banks), double
buffered. Gate order is (f, i, o, g). The g block is pre-scaled by 2 in the
weights so tanh(g) = 2*sigmoid(2g) - 1 and a single fused sigmoid covers
i, o, g; sigmoid(f) is a separate op so the cell-state update can start
early. The per-gate bias is folded into the x-projection matmul via a
constant ones row (K=3). In the forecast phase fc_b is folded into that
bias (b + W_ih @ fc_b) and added back to the returned y on the host.
"""

import sys

for _p in ("/opt/trn_rl_repo",):
    if _p not in sys.path:
        sys.path.insert(0, _p)

import numpy as np

import concourse.bass as bass
import concourse.bacc as bacc
import concourse.mybir as mybir
import concourse.tile as tile
from concourse.bass_utils import run_bass_kernel_spmd

# Problem constants (hardcoded per spec).
B_TOT = 4096
T = 512
IN = 2
H = 128
OUT = 2
FUT = 50
NCORES = 8
B = B_TOT // NCORES  # 512 batch rows per core
HB = B // 2  # half-batch for pipelining
NSTEPS = T + FUT - 1  # last cell step feeding a y (the final cell is unused)

F32 = mybir.dt.float32
F32R = mybir.dt.float32r
AF = mybir.ActivationFunctionType
ALU = mybir.AluOpType

X_PREFETCH = 8

# Experiment knobs (sim-guided tuning; defaults = shipped config).
VARIANT = {
    "fuse_sigma_all": False,  # one sigmoid over all 4 gates vs sigma_f split
    "stage_order": False,     # stage-interleaved emission vs per-half blocks
    "t1_engine": "gpsimd",    # gpsimd | vector
}

# Gate order in PSUM/weights: f, i, o, g. PyTorch row order in W_ih/W_hh
# is i, f, g, o.
_TORCH_SLOT = {"i": 0, "f": 1, "g": 2, "o": 3}
_GATES = ("f", "i", "o", "g")


def _build_nc(nsteps=NSTEPS, dump_state=False, timing_reps=1):
    nc = bacc.Bacc("TRN2", target_bir_lowering=False)

    x_aug = nc.dram_tensor("x_aug", [T, 3, B], F32R, kind="ExternalInput")
    w_hh = nc.dram_tensor("w_hh", [H, 4, H], F32R, kind="ExternalInput")
    w_ih_e = nc.dram_tensor("w_ih_e", [3, 4, H], F32R, kind="ExternalInput")
    w_ih_f = nc.dram_tensor("w_ih_f", [3, 4, H], F32R, kind="ExternalInput")
    fc_wt = nc.dram_tensor("fc_wt", [H, OUT], F32R, kind="ExternalInput")
    ones3 = nc.dram_tensor("ones3", [3, B], F32R, kind="ExternalInput")
    y_out = nc.dram_tensor("y_out", [OUT, FUT, B], F32R, kind="ExternalOutput")
    if dump_state:
        h_out = nc.dram_tensor("h_out", [H, B], F32R, kind="ExternalOutput")
        c_out = nc.dram_tensor("c_out", [H, B], F32, kind="ExternalOutput")

    with tile.TileContext(nc) as tc:
        with (
            tc.tile_pool(name="consts", bufs=1) as consts,
            tc.tile_pool(name="state", bufs=1) as state,
            tc.tile_pool(name="xpool", bufs=X_PREFETCH) as xpool,
            tc.tile_pool(name="psum", bufs=2, space="PSUM") as psum,
        ):
            w_hh_sb = consts.tile([H, 4, H], F32R)
            nc.sync.dma_start(out=w_hh_sb, in_=w_hh[:, :, :])
            w_ih_e_sb = consts.tile([3, 4, H], F32R)
            nc.sync.dma_start(out=w_ih_e_sb, in_=w_ih_e[:, :, :])
            w_ih_f_sb = consts.tile([3, 4, H], F32R)
            nc.sync.dma_start(out=w_ih_f_sb, in_=w_ih_f[:, :, :])
            fc_wt_sb = consts.tile([H, OUT], F32R)
            nc.sync.dma_start(out=fc_wt_sb, in_=fc_wt[:, :])

            h_sb = state.tile([H, B], F32R)
            c_sb = state.tile([H, B], F32)
            tc_sb = state.tile([H, B], F32)
            sig_sb = state.tile([H, 4, B], F32)
            t1_sb = state.tile([H, B], F32)
            t2_sb = state.tile([H, B], F32)
            y_stage = state.tile([3, B], F32R)

            nc.vector.memset(c_sb, 0.0)
            # Row 2 is the constant ones row (bias trick); rows 0-1 are
            # overwritten by the forecast y copy before any read. DMA init
            # because engines can't memset a float32r tile. h_sb needs no
            # init: step 0 skips the h-matmuls (h0 == 0) and h_sb is first
            # written at the end of step 0.
            nc.sync.dma_start(out=y_stage, in_=ones3[:, :])

            x_tiles = {}

            def x_matmuls(gt, t):
                """Emit the K=3 input-projection matmuls (start=True) for
                step t into gates tile gt; includes the bias via ones row."""
                if t < T:
                    rhs = x_tiles.pop(t)
                    lhs = w_ih_e_sb
                else:
                    rhs = y_stage
                    lhs = w_ih_f_sb
                for g in range(4):
                    nc.tensor.matmul(
                        gt[:, g, :],
                        lhsT=lhs[:, g, :],
                        rhs=rhs[:, :],
                        start=True,
                        # Step 0 skips the h-matmuls, so its group ends here.
                        stop=(t == 0),
                        skip_group_check=True,
                    )

            def emit_steps():
                for t in range(min(X_PREFETCH, T)):
                    xt = xpool.tile([3, B], F32R, name=f"x_{t}", tag="x")
                    nc.sync.dma_start(out=xt, in_=x_aug[t, :, :])
                    x_tiles[t] = xt

                gt = psum.tile([H, 4, B], F32, name="gates", tag="gates")
                x_matmuls(gt, 0)
                for t in range(nsteps):
                    emit_step(t, gt)
                    gt = step_next[0]

            step_next = [None]

            HALVES = [slice(0, HB), slice(HB, B)]

            def emit_half_tail(t, cur, sl):
                """ACT/GPS/DVE tail for one half-batch stream."""
                if VARIANT["fuse_sigma_all"]:
                    nc.scalar.activation(
                        sig_sb[:, 0:4, sl], cur[:, 0:4, sl], AF.Sigmoid
                    )
                else:
                    nc.scalar.activation(sig_sb[:, 0, sl], cur[:, 0, sl], AF.Sigmoid)
                    nc.scalar.activation(
                        sig_sb[:, 1:4, sl], cur[:, 1:4, sl], AF.Sigmoid
                    )
                _t1eng = nc.gpsimd if VARIANT["t1_engine"] == "gpsimd" else nc.vector
                _t1eng.tensor_mul(t1_sb[:, sl], sig_sb[:, 0, sl], c_sb[:, sl])
                nc.vector.scalar_tensor_tensor(
                    t2_sb[:, sl],
                    in0=sig_sb[:, 3, sl],
                    scalar=0.5,
                    in1=sig_sb[:, 1, sl],
                    op0=ALU.subtract,
                    op1=ALU.mult,
                )
                nc.vector.scalar_tensor_tensor(
                    c_sb[:, sl],
                    in0=t2_sb[:, sl],
                    scalar=2.0,
                    in1=t1_sb[:, sl],
                    op0=ALU.mult,
                    op1=ALU.add,
                )
                nc.scalar.activation(tc_sb[:, sl], c_sb[:, sl], AF.Tanh)
                nc.vector.tensor_mul(h_sb[:, sl], sig_sb[:, 2, sl], tc_sb[:, sl])

            def emit_step_perhalf(t, cur):
                for sl in HALVES:
                    if t > 0:
                        for g in range(4):
                            nc.tensor.matmul(
                                cur[:, g, sl],
                                lhsT=w_hh_sb[:, g, :],
                                rhs=h_sb[:, sl],
                                start=False,
                                stop=True,
                                skip_group_check=True,
                            )
                    emit_half_tail(t, cur, sl)

            def emit_step(t, cur):
                if not VARIANT["stage_order"]:
                    emit_step_perhalf(t, cur)
                    emit_step_common(t, cur)
                    return
                # Stage-ordered emission: each engine's queue alternates
                # between the two half-batch streams so the in-order queues
                # never serialize one half's chain behind the other's.
                # Gate pre-activations: accumulate W_hh.T @ h. Step 0 has
                # h == 0 and skips the h-matmuls (PSUM holds x-part only).
                if t > 0:
                    for sl in HALVES:
                        for g in range(4):
                            nc.tensor.matmul(
                                cur[:, g, sl],
                                lhsT=w_hh_sb[:, g, :],
                                rhs=h_sb[:, sl],
                                start=False,
                                stop=True,
                                skip_group_check=True,
                            )
                if VARIANT["fuse_sigma_all"]:
                    for sl in HALVES:
                        nc.scalar.activation(
                            sig_sb[:, 0:4, sl], cur[:, 0:4, sl], AF.Sigmoid
                        )
                else:
                    for sl in HALVES:
                        nc.scalar.activation(
                            sig_sb[:, 0, sl], cur[:, 0, sl], AF.Sigmoid
                        )
                        nc.scalar.activation(
                            sig_sb[:, 1:4, sl], cur[:, 1:4, sl], AF.Sigmoid
                        )
                _t1eng = nc.gpsimd if VARIANT["t1_engine"] == "gpsimd" else nc.vector
                for sl in HALVES:
                    _t1eng.tensor_mul(t1_sb[:, sl], sig_sb[:, 0, sl], c_sb[:, sl])
                for sl in HALVES:
                    nc.vector.scalar_tensor_tensor(
                        t2_sb[:, sl],
                        in0=sig_sb[:, 3, sl],
                        scalar=0.5,
                        in1=sig_sb[:, 1, sl],
                        op0=ALU.subtract,
                        op1=ALU.mult,
                    )
                    nc.vector.scalar_tensor_tensor(
                        c_sb[:, sl],
                        in0=t2_sb[:, sl],
                        scalar=2.0,
                        in1=t1_sb[:, sl],
                        op0=ALU.mult,
                        op1=ALU.add,
                    )
                for sl in HALVES:
                    nc.scalar.activation(tc_sb[:, sl], c_sb[:, sl], AF.Tanh)
                for sl in HALVES:
                    nc.vector.tensor_mul(h_sb[:, sl], sig_sb[:, 2, sl], tc_sb[:, sl])
                emit_step_common(t, cur)

            def emit_step_common(t, cur):
                # Prefetch x for a future step.
                pf = t + X_PREFETCH
                if pf < T:
                    xt = xpool.tile([3, B], F32R, name=f"x_{pf}", tag="x")
                    nc.sync.dma_start(out=xt, in_=x_aug[pf, :, :])
                    x_tiles[pf] = xt

                nxt = None
                if t + 1 < nsteps or t >= T - 1:
                    nxt = psum.tile([H, 4, B], F32, name="gates", tag="gates")
                if t >= T - 1:
                    # y_j = fc_w @ h (fc_b folded into forecast bias / added
                    # back on host). Lands in the next gates tile's f bank,
                    # which the start=True x-matmul later overwrites.
                    j = t - (T - 1)
                    nc.tensor.matmul(
                        nxt[0:OUT, 0, :],
                        lhsT=fc_wt_sb[:, :],
                        rhs=h_sb[:, :],
                        start=True,
                        stop=True,
                        skip_group_check=True,
                    )
                    nc.vector.tensor_copy(y_stage[0:OUT, :], nxt[0:OUT, 0, :])
                    if j < FUT:
                        nc.sync.dma_start(out=y_out[:, j, :], in_=y_stage[0:OUT, :])
                if t + 1 < nsteps:
                    x_matmuls(nxt, t + 1)
                step_next[0] = nxt

            if timing_reps > 1:
                with tc.For_i(0, timing_reps, 1):
                    emit_steps()
            else:
                emit_steps()

            if dump_state:
                nc.sync.dma_start(out=h_out[:, :], in_=h_sb[:, :])
                nc.sync.dma_start(out=c_out[:, :], in_=c_sb[:, :])

    nc.compile()
    return nc


_NC_CACHE = None


def _get_nc():
    global _NC_CACHE
    if _NC_CACHE is None:
        _NC_CACHE = _build_nc()
    return _NC_CACHE


def _prep_weights(W_ih, W_hh, b_ih, b_hh, fc_w, fc_b):
    """Host-side weight repacking into the kernel's gate order (f,i,o,g),
    with the g block pre-scaled by 2 and biases folded in."""

    def blocks(mat):
        # mat: [4H, ...] in torch order i,f,g,o -> dict gate -> [H, ...]
        return {g: mat[_TORCH_SLOT[g] * H : (_TORCH_SLOT[g] + 1) * H] for g in _TORCH_SLOT}

    wih_b = blocks(W_ih)  # [H, IN] each
    whh_b = blocks(W_hh)  # [H, H] each
    bias = b_ih + b_hh
    bias_b = blocks(bias)  # [H] each
    bias_fc_full = bias + W_ih @ fc_b
    bias_fc_b = blocks(bias_fc_full)

    w_hh_arr = np.empty((H, 4, H), np.float32)
    w_ih_e_arr = np.empty((3, 4, H), np.float32)
    w_ih_f_arr = np.empty((3, 4, H), np.float32)
    for gi, g in enumerate(_GATES):
        s = 2.0 if g == "g" else 1.0
        w_hh_arr[:, gi, :] = s * whh_b[g].T
        w_ih_e_arr[0:IN, gi, :] = s * wih_b[g].T
        w_ih_e_arr[2, gi, :] = s * bias_b[g]
        w_ih_f_arr[0:IN, gi, :] = s * wih_b[g].T
        w_ih_f_arr[2, gi, :] = s * bias_fc_b[g]
    fc_wt_arr = np.ascontiguousarray(fc_w.T, dtype=np.float32)  # [H, OUT]
    return w_hh_arr, w_ih_e_arr, w_ih_f_arr, fc_wt_arr


def kernel(x, W_ih, W_hh, b_ih, b_hh, fc_w, fc_b):
    x = np.asarray(x, np.float32)
    W_ih = np.asarray(W_ih, np.float32)
    W_hh = np.asarray(W_hh, np.float32)
    b_ih = np.asarray(b_ih, np.float32)
    b_hh = np.asarray(b_hh, np.float32)
    fc_w = np.asarray(fc_w, np.float32)
    fc_b = np.asarray(fc_b, np.float32)

    w_hh_arr, w_ih_e_arr, w_ih_f_arr, fc_wt_arr = _prep_weights(
        W_ih, W_hh, b_ih, b_hh, fc_w, fc_b
    )

    in_maps = []
    for k in range(NCORES):
        xs = x[k * B : (k + 1) * B]  # [B, T, IN]
        x_aug = np.empty((T, 3, B), np.float32)
        x_aug[:, 0:IN, :] = xs.transpose(1, 2, 0)
        x_aug[:, 2, :] = 1.0
        in_maps.append(
            {
                "x_aug": np.ascontiguousarray(x_aug),
                "w_hh": w_hh_arr,
                "w_ih_e": w_ih_e_arr,
                "w_ih_f": w_ih_f_arr,
                "fc_wt": fc_wt_arr,
                "ones3": np.ones((3, B), np.float32),
            }
        )

    nc = _get_nc()
    res = run_bass_kernel_spmd(nc, in_maps, core_ids=list(range(NCORES)))

    out = np.empty((B_TOT, FUT * OUT), np.float32)
    bias_tile = np.tile(fc_b, FUT).astype(np.float32)
    for k in range(NCORES):
        ys = res.results[k]["y_out"]  # [OUT, FUT, B]
        out[k * B : (k + 1) * B] = ys.transpose(2, 1, 0).reshape(B, FUT * OUT)
    out += bias_tile
    return out

